# revision 5
# baseline (speedup 1.0000x reference)
"""ActiveShiftLayer Trainium2 kernel.

out[n,c,h,w] = bilinear sample of x[n,c, h+alpha_c, w+beta_c], zero outside.

Since alpha,beta in [-1,1), floor in {-1,0}; the bilinear sample is a
separable 3-tap convolution along W then H with per-channel tap weights:
    tmp[h,w] = sum_dx wh[c,dx] * x[h, w+dx]      (dx in {-1,0,1}, zero pad)
    out[h,w] = sum_dy wv[c,dy] * tmp[h+dy, w]    (dy in {-1,0,1}, zero pad)
Weights (6 per channel) are computed on host from shift_param [C,2] and
passed as a tiny extra input.

Layout per core: data-parallel over batch (N=32 -> 4 per core). Each tile:
128 channels on partitions, one 56x56 plane on the free dim. W-padded input
tile XP (rows of 58: [0, x_row, 0]) lets the 3 W-taps be constant free-dim
offsets; H-padded intermediate HT (58 rows of 56 with zero first/last row)
does the same for H-taps.
"""

import os
import numpy as np

N, C, H, W = 32, 256, 56, 56
NCORES = 8
NSH = N // NCORES  # batches per core
P = 128
CB = C // P        # channel blocks
HW = H * W         # 3136
WROW = W + 2       # 58: padded row length in XP
XPF = H * WROW     # 3248
HTF = (H + 2) * W  # 3248

_CACHE = {}


def _build_nc():
    import concourse.bacc as bacc
    import concourse.mybir as mybir
    import concourse.tile as tile

    f32 = mybir.dt.float32
    mult = mybir.AluOpType.mult
    add = mybir.AluOpType.add
    act_copy = mybir.ActivationFunctionType.Copy

    nc = bacc.Bacc()
    xs = nc.dram_tensor("xs", [NSH, C, H, W], f32, kind="ExternalInput")
    wt = nc.dram_tensor("wt", [C, 6], f32, kind="ExternalInput")
    ys = nc.dram_tensor("ys", [NSH, C, H, W], f32, kind="ExternalOutput")

    with tile.TileContext(nc) as tc:
        with tc.tile_pool(name="wp", bufs=1) as wp, \
             tc.tile_pool(name="xpp", bufs=3) as xpool, \
             tc.tile_pool(name="htp", bufs=2) as hpool, \
             tc.tile_pool(name="outp", bufs=3) as opool:
            wsb = []
            for cb in range(CB):
                wtile = wp.tile([P, 6], f32, tag=f"w{cb}")
                nc.sync.dma_start(wtile[:], wt[cb * P:(cb + 1) * P, :])
                wsb.append(wtile)

            for n in range(NSH):
                for cb in range(CB):
                    wv = wsb[cb]
                    cs = slice(cb * P, (cb + 1) * P)

                    XP = xpool.tile([P, XPF], f32)
                    XPv = XP[:].rearrange("p (r c) -> p r c", c=WROW)
                    nc.gpsimd.memset(XPv[:, :, 0], 0.0)
                    nc.gpsimd.memset(XPv[:, :, W + 1], 0.0)
                    nc.sync.dma_start(XPv[:, :, 1:W + 1], xs[n, cs, :, :])

                    HT = hpool.tile([P, HTF], f32)
                    nc.gpsimd.memset(HT[:, 0:W], 0.0)
                    nc.gpsimd.memset(HT[:, (H + 1) * W:], 0.0)
                    ctr = HT[:, W:(H + 1) * W].rearrange("p (r c) -> p r c", c=W)

                    # horizontal pass: taps at XP col offsets 0,1,2
                    nc.scalar.activation(ctr, XPv[:, :, 1:W + 1], act_copy,
                                         scale=wv[:, 1:2])
                    nc.vector.scalar_tensor_tensor(
                        ctr, XPv[:, :, 0:W], wv[:, 0:1], ctr, op0=mult, op1=add)
                    nc.vector.scalar_tensor_tensor(
                        ctr, XPv[:, :, 2:W + 2], wv[:, 2:3], ctr, op0=mult, op1=add)

                    # vertical pass: taps at HT row offsets 0,1,2
                    OUT = opool.tile([P, HW], f32)
                    nc.scalar.activation(OUT[:], HT[:, W:W + HW], act_copy,
                                         scale=wv[:, 4:5])
                    nc.vector.scalar_tensor_tensor(
                        OUT[:], HT[:, 0:HW], wv[:, 3:4], OUT[:], op0=mult, op1=add)
                    nc.vector.scalar_tensor_tensor(
                        OUT[:], HT[:, 2 * W:2 * W + HW], wv[:, 5:6], OUT[:],
                        op0=mult, op1=add)

                    nc.sync.dma_start(
                        ys[n, cs, :, :],
                        OUT[:].rearrange("p (h w) -> p h w", w=W))
    nc.finalize()
    return nc


def _tap_weights(shift):
    """Per-channel 3-tap weights over offsets {-1,0,1} for shift in [-1,1)."""
    f = np.floor(shift)
    t = (shift - f).astype(np.float32)
    assert np.all((f == -1) | (f == 0)), "shift outside [-1,1) unsupported"
    w_m1 = np.where(f == -1, 1 - t, 0).astype(np.float32)
    w_0 = np.where(f == -1, t, 1 - t).astype(np.float32)
    w_p1 = np.where(f == 0, t, 0).astype(np.float32)
    return w_m1, w_0, w_p1


def _install_trace_shim():
    """Dev-only: register the NTFF profile hook this container's antenv lacks,
    and stub out the artifact upload (zero-egress container)."""
    import sys
    import types

    try:
        from antenv.axon_hooks import get_axon_ntff_profile_hook  # noqa: F401
    except ImportError:
        from trn_agent_boot.trn_boot import _ntff_profile_via_ctypes

        hook = _ntff_profile_via_ctypes("/opt/axon/libaxon_pjrt.so")
        mod = types.ModuleType("antenv.axon_hooks")
        mod.get_axon_ntff_profile_hook = lambda: hook
        mod.set_axon_ntff_profile_hook = lambda h: None
        import antenv

        sys.modules["antenv.axon_hooks"] = mod
        antenv.axon_hooks = mod

    import concourse.bass_utils as bu

    bu.upload_artifacts = lambda tmpdir: tmpdir


def kernel(x, shift_param):
    from concourse.bass_utils import run_bass_kernel_spmd

    x = np.ascontiguousarray(np.asarray(x, dtype=np.float32))
    sp = np.asarray(shift_param, dtype=np.float32)
    assert x.shape == (N, C, H, W)

    wh_m1, wh_0, wh_p1 = _tap_weights(sp[:, 1])  # beta: W shift
    wv_m1, wv_0, wv_p1 = _tap_weights(sp[:, 0])  # alpha: H shift
    wt = np.stack([wh_m1, wh_0, wh_p1, wv_m1, wv_0, wv_p1], axis=1)
    wt = np.ascontiguousarray(wt.astype(np.float32))

    if "nc" not in _CACHE:
        _CACHE["nc"] = _build_nc()
    nc = _CACHE["nc"]

    in_maps = [{"xs": x[i * NSH:(i + 1) * NSH], "wt": wt} for i in range(NCORES)]
    trace = os.environ.get("ASL_TRACE") == "1"
    if trace:
        _install_trace_shim()
    res = run_bass_kernel_spmd(nc, in_maps, list(range(NCORES)), trace=trace)
    if trace:
        print(f"HW exec time: {res.exec_time_ns} ns")
        _CACHE["last_result"] = res
    out = np.concatenate([r["ys"] for r in res.results], axis=0)
    return out


# revision 9
# speedup vs baseline: 1.1417x; 1.1417x over previous
"""ActiveShiftLayer Trainium2 kernel.

out[n,c,h,w] = bilinear sample of x[n,c, h+alpha_c, w+beta_c], zero outside.

alpha,beta in [-1,1) => floor in {-1,0}; the bilinear sample is a separable
3-tap convolution along W then H with per-channel tap weights:
    tmp[h,w] = sum_dx wh[c,dx] * x[h, w+dx]      (dx in {-1,0,1}, zero pad)
    out[h,w] = sum_dy wv[c,dy] * tmp[h+dy, w]    (dy in {-1,0,1}, zero pad)
Weights are computed on host from shift_param [C,2] and passed as extra
inputs.

Data-parallel over batch (N=32 -> 4 per core); per core 8 tiles of
[128 channels (partitions), 56*56 plane (free dim)].

Per-tile schedule (f32 end-to-end except the H-stage products in float32r):
- contiguous DMA load into X[128, 1+3136+1] (1-elem guard pads)
- H-stage on TensorE: per 512-col chunk, 3 accumulating float32r matmuls
  with diagonal weight matrices (diag applies per-channel tap weight); flat
  taps at offsets {-1,0,+1} wrap across row boundaries, fixed later
- ScalarE copies PSUM -> SBUF HT center (rows 1..56 of a 58-row buffer
  whose first/last rows are zeroed)
- GPSIMD fixes the two wrapped columns: tmp[h][0] -= wh_m1*x[h-1][55],
  tmp[h][55] -= wh_p1*x[h+1][0] (strided 56-elem scalar_tensor_tensor)
- V-stage: ScalarE center tap (activation scale), VectorE outer taps
  (scalar_tensor_tensor accumulate), all exact f32
- contiguous DMA store
"""

import os
import numpy as np

N, C, H, W = 32, 256, 56, 56
NCORES = 8
NSH = N // NCORES  # batches per core
P = 128
CB = C // P        # channel blocks
HW = H * W         # 3136
XF = HW + 2        # X tile free size (guard pad at 0 and HW+1)
HTF = (H + 2) * W  # 3248
CHUNK = 512
NCHUNK = (HW + CHUNK - 1) // CHUNK  # 7 (last chunk = 64)

_CACHE = {}


def _build_nc():
    import concourse.bacc as bacc
    import concourse.mybir as mybir
    import concourse.tile as tile

    f32 = mybir.dt.float32
    f32r = mybir.dt.float32r
    mult = mybir.AluOpType.mult
    add = mybir.AluOpType.add
    act_copy = mybir.ActivationFunctionType.Copy

    nc = bacc.Bacc()
    xs = nc.dram_tensor("xs", [NSH, C, H, W], f32, kind="ExternalInput")
    # wd[cb, tap] = diag(wh_tap) for channels cb*128..cb*128+127
    wd = nc.dram_tensor("wd", [CB, 3, P, P], f32, kind="ExternalInput")
    # wv[cb] columns: [wv_m1, wv_0, wv_p1, -wh_m1, -wh_p1]
    wv = nc.dram_tensor("wv", [CB, P, 5], f32, kind="ExternalInput")
    ys = nc.dram_tensor("ys", [NSH, C, H, W], f32, kind="ExternalOutput")

    with tile.TileContext(nc) as tc:
        with tc.tile_pool(name="wp", bufs=1) as wp, \
             tc.tile_pool(name="xp", bufs=3) as xpool, \
             tc.tile_pool(name="ht", bufs=2) as hpool, \
             tc.tile_pool(name="op", bufs=3) as opool, \
             tc.tile_pool(name="ps", bufs=1, space="PSUM") as ppool:

            wdt = []
            wvt = []
            for cb in range(CB):
                t = wp.tile([P, 3 * P], f32r, tag=f"wd{cb}")
                nc.gpsimd.dma_start(
                    t[:].rearrange("p (t q) -> p t q", t=3),
                    wd[cb].rearrange("t p q -> p t q"))
                wdt.append(t)
                v = wp.tile([P, 5], f32, tag=f"wv{cb}")
                nc.sync.dma_start(v[:], wv[cb])
                wvt.append(v)

            for n in range(NSH):
                for cb in range(CB):
                    wvc = wvt[cb]
                    cs = slice(cb * P, (cb + 1) * P)

                    X = xpool.tile([P, XF], f32r)
                    nc.gpsimd.memset(X[:, 0:1].bitcast(f32), 0.0)
                    nc.gpsimd.memset(X[:, XF - 1:XF].bitcast(f32), 0.0)
                    nc.gpsimd.dma_start(X[:, 1:1 + HW], xs[n, cs, :, :])

                    PS = ppool.tile([P, HW], f32, tag="ps")
                    for j in range(NCHUNK):
                        c0 = j * CHUNK
                        cn = min(CHUNK, HW - c0)
                        for tap in range(3):
                            nc.tensor.matmul(
                                PS[:, c0:c0 + cn],
                                wdt[cb][:, tap * P:(tap + 1) * P],
                                X[:, c0 + tap:c0 + tap + cn],
                                start=(tap == 0), stop=(tap == 2))

                    HT = hpool.tile([P, HTF], f32)
                    nc.gpsimd.memset(HT[:, 0:W], 0.0)
                    nc.gpsimd.memset(HT[:, HTF - W:], 0.0)
                    ctr = HT[:, W:W + HW]
                    nc.scalar.activation(ctr, PS[:], act_copy)

    # wrapped-column fixups (strided, per channel):
                    #   tmp[h][0]  -= wh_m1 * x[h-1][55]   (x[-1][55] := guard X[0] = 0)
                    #   tmp[h][55] -= wh_p1 * x[h+1][0]    (h=55 term is 0: guard X[3137])
                    ctr2 = ctr.rearrange("p (h w) -> p h w", w=W)
                    col0 = ctr2[:, :, 0]
                    col55 = ctr2[:, 0:H - 1, W - 1]
                    # x[h-1][55] = X[56h]; x[h+1][0] = X[1 + 56(h+1)]
                    xg0 = X[:, 0:HW].bitcast(f32).rearrange(
                        "p (h w) -> p h w", w=W)[:, :, 0]
                    xg55 = X[:, 1:1 + HW].bitcast(f32).rearrange(
                        "p (h w) -> p h w", w=W)[:, 1:H, 0]
                    nc.vector.scalar_tensor_tensor(
                        col0, xg0, wvc[:, 3:4], col0, op0=mult, op1=add)
                    nc.vector.scalar_tensor_tensor(
                        col55, xg55, wvc[:, 4:5], col55, op0=mult, op1=add)

                    OUT = opool.tile([P, HW], f32)
                    nc.scalar.activation(OUT[:], ctr, act_copy, scale=wvc[:, 1:2])
                    nc.vector.scalar_tensor_tensor(
                        OUT[:], HT[:, 0:HW], wvc[:, 0:1], OUT[:], op0=mult, op1=add)
                    nc.vector.scalar_tensor_tensor(
                        OUT[:], HT[:, 2 * W:2 * W + HW], wvc[:, 2:3], OUT[:],
                        op0=mult, op1=add)

                    nc.sync.dma_start(
                        ys[n, cs, :, :],
                        OUT[:].rearrange("p (h w) -> p h w", w=W))
    nc.finalize()
    return nc


def _tap_weights(shift):
    """Per-channel 3-tap weights over offsets {-1,0,1} for shift in [-1,1)."""
    f = np.floor(shift)
    t = (shift - f).astype(np.float32)
    assert np.all((f == -1) | (f == 0)), "shift outside [-1,1) unsupported"
    w_m1 = np.where(f == -1, 1 - t, 0).astype(np.float32)
    w_0 = np.where(f == -1, t, 1 - t).astype(np.float32)
    w_p1 = np.where(f == 0, t, 0).astype(np.float32)
    return w_m1, w_0, w_p1


def _host_weights(sp):
    wh_m1, wh_0, wh_p1 = _tap_weights(sp[:, 1])  # beta: W shift
    wv_m1, wv_0, wv_p1 = _tap_weights(sp[:, 0])  # alpha: H shift
    wd = np.zeros((CB, 3, P, P), np.float32)
    for cb in range(CB):
        cs = slice(cb * P, (cb + 1) * P)
        for t, w in enumerate((wh_m1, wh_0, wh_p1)):
            wd[cb, t] = np.diag(w[cs])
    wv = np.stack([wv_m1, wv_0, wv_p1, -wh_m1, -wh_p1], axis=1).astype(np.float32)
    wv = np.ascontiguousarray(wv.reshape(CB, P, 5))
    return np.ascontiguousarray(wd), wv


def _install_trace_shim():
    """Dev-only: register the NTFF profile hook this container's antenv lacks,
    and stub out the artifact upload (zero-egress container)."""
    import sys
    import types

    try:
        from antenv.axon_hooks import get_axon_ntff_profile_hook  # noqa: F401
    except ImportError:
        from trn_agent_boot.trn_boot import _ntff_profile_via_ctypes

        hook = _ntff_profile_via_ctypes("/opt/axon/libaxon_pjrt.so")
        mod = types.ModuleType("antenv.axon_hooks")
        mod.get_axon_ntff_profile_hook = lambda: hook
        mod.set_axon_ntff_profile_hook = lambda h: None
        import antenv

        sys.modules["antenv.axon_hooks"] = mod
        antenv.axon_hooks = mod

    import concourse.bass_utils as bu

    bu.upload_artifacts = lambda tmpdir: tmpdir


def kernel(x, shift_param):
    from concourse.bass_utils import run_bass_kernel_spmd

    x = np.ascontiguousarray(np.asarray(x, dtype=np.float32))
    sp = np.asarray(shift_param, dtype=np.float32)
    assert x.shape == (N, C, H, W)

    wd, wv = _host_weights(sp)

    if "nc" not in _CACHE:
        _CACHE["nc"] = _build_nc()
    nc = _CACHE["nc"]

    in_maps = [{"xs": x[i * NSH:(i + 1) * NSH], "wd": wd, "wv": wv}
               for i in range(NCORES)]
    trace = os.environ.get("ASL_TRACE") == "1"
    if trace:
        _install_trace_shim()
    res = run_bass_kernel_spmd(nc, in_maps, list(range(NCORES)), trace=trace)
    if trace:
        print(f"HW exec time: {res.exec_time_ns} ns")
        _CACHE["last_result"] = res
    out = np.concatenate([r["ys"] for r in res.results], axis=0)
    return out
